# revision 1
# baseline (speedup 1.0000x reference)
"""CompressionTransformerLayer on 8 TRN2 NeuronCores (Bass/Tile).

Sharding: tensor-parallel by heads (16 heads -> 2 per core).
 - self-attention: each core computes its 2 heads; AllGather of head outputs
   (bf16, 0.25MB/core); replicated output projection + residual -> x.
 - cross-attention: each core projects K/V for its 2 heads over the FULL
   context (context replicated in HBM, streamed in 512-token chunks),
   transposed-scores softmax (exp on ScalarE, denominator via an appended
   ones-column in V), local normalization, AllGather of head outputs.
 - tail: co-projection + residual replicated (cheap); each core then selects
   ITS 128 tokens with a per-core one-hot matmul (SPMD-safe core identity via
   data), runs LN3 + FFN on them, and outputs its [128, 1024] slice; the host
   concatenates.
All matmuls bf16 with fp32 PSUM accumulation; LN/softmax/residuals in fp32.
Residual streams x/x3 bounce through DRAM to keep SBUF pressure low.
"""
import sys
sys.path.insert(0, "/opt/trn_rl_repo")
sys.path.insert(0, "/root/.axon_site")

import contextlib
import numpy as np

import concourse.bass as bass
import concourse.mybir as mybir
import concourse.tile as tile
from concourse import bacc
from concourse.bass_utils import run_bass_kernel_spmd

f32, bf16 = mybir.dt.float32, mybir.dt.bfloat16
AF = mybir.ActivationFunctionType
ALU = mybir.AluOpType
BF16NP = mybir.dt.np(bf16)

D, H, HD, DFF = 1024, 16, 64, 4096
B, Q, S = 4, 256, 8192
NC = 8
T = B * Q            # 1024 flattened query tokens
HPC = H // NC        # 2 heads per core
FO = D // 128        # 8 feature tiles
SCH = 512            # context chunk (tokens)
NSC = S // SCH       # 16 chunks per batch element
EPS = 1e-5

_CACHE = {}


def _build():
    nc = bacc.Bacc("TRN2", target_bir_lowering=False, debug=False,
                   enable_asserts=True, num_devices=NC)

    def din(name, shape, dt=bf16):
        return nc.dram_tensor(name, shape, dt, kind="ExternalInput").ap()

    queries = din("queries", [T, D], f32)
    ctx = din("ctx", [D, B * S], bf16)
    ln1g = din("ln1g", [128, FO], f32); ln1b = din("ln1b", [128, FO], f32)
    ln2g = din("ln2g", [128, FO], f32); ln2b = din("ln2b", [128, FO], f32)
    ln3g = din("ln3g", [128, FO], f32); ln3b = din("ln3b", [128, FO], f32)
    wq = din("wq", [D, 128]); wk = din("wk", [D, 128]); wv = din("wv", [D, 128])
    bq = din("bq", [128], f32); bk = din("bk", [128], f32); bv = din("bv", [128], f32)
    saow = din("saow", [D, D]); saob = din("saob", [D], f32)
    cqw = din("cqw", [D, 128]); ckw = din("ckw", [D, 128]); cvw = din("cvw", [D, 128])
    cqb = din("cqb", [128], f32); ckb = din("ckb", [128], f32); cvb = din("cvb", [128], f32)
    cow = din("cow", [D, D]); cob = din("cob", [D], f32)
    w1 = din("w1", [D, DFF]); b1 = din("b1", [DFF], f32)
    w2 = din("w2", [DFF, D]); b2 = din("b2", [D], f32)
    # per-core one-hot token-selection: psel[p, tt, j] = 1 iff tt == core, p == j
    psel = din("psel", [128, FO, 128], f32)

    out = nc.dram_tensor("out", [128, D], f32, kind="ExternalOutput").ap()
    import os
    DBG = bool(os.environ.get("BASSDBG"))
    dbg = {}
    if DBG:
        for nm, shp, dt in [("dbg_qn", [T, D], bf16), ("dbg_qs", [128, T], bf16),
                            ("dbg_ks", [128, T], bf16), ("dbg_vs", [128, T], bf16),
                            ("dbg_oself", [128, T], bf16), ("dbg_x", [T, D], f32),
                            ("dbg_q2", [128, T], bf16), ("dbg_ocross", [128, T], bf16),
                            ("dbg_x3", [T, D], f32), ("dbg_x3own", [128, D], f32),
                            ("dbg_h", [128, DFF], bf16), ("dbg_ag1", [NC, 128, T], bf16)]:
            dbg[nm] = nc.dram_tensor(nm, shp, dt, kind="ExternalOutput").ap()

    ctx_r = ctx.rearrange("(fo fi) t -> fi fo t", fi=128)
    w1_r = w1.rearrange("(fo fi) n -> fi fo n", fi=128)
    w2_r = w2.rearrange("(dg fi) n -> fi dg n", fi=128)
    saow_r = saow.rearrange("(fo fi) n -> fi fo n", fi=128)
    cow_r = cow.rearrange("(fo fi) n -> fi fo n", fi=128)

    with tile.TileContext(nc) as tc:
        with contextlib.ExitStack() as ctxs:
            const = ctxs.enter_context(tc.tile_pool(name="const", bufs=1))
            big = ctxs.enter_context(tc.tile_pool(name="big", bufs=1))
            fm = ctxs.enter_context(tc.tile_pool(name="fm", bufs=1))
            sb = ctxs.enter_context(tc.tile_pool(name="sb", bufs=3))
            stream = ctxs.enter_context(tc.tile_pool(name="stream", bufs=2))
            rstream = ctxs.enter_context(tc.tile_pool(name="rstream", bufs=3))
            wstream = ctxs.enter_context(tc.tile_pool(name="wstream", bufs=2))
            dram = ctxs.enter_context(tc.tile_pool(name="dram", bufs=1, space="DRAM"))
            psA = ctxs.enter_context(tc.tile_pool(name="psA", bufs=4, space="PSUM"))
            psS = ctxs.enter_context(tc.tile_pool(name="psS", bufs=2, space="PSUM"))
            psO = ctxs.enter_context(tc.tile_pool(name="psO", bufs=2, space="PSUM"))

            def ldconst(ap_, shape, dt, name):
                t = const.tile(shape, dt, tag=name)
                nc.sync.dma_start(t[:], ap_)
                return t

            wq_sb = ldconst(wq.rearrange("(fo fi) o -> fi fo o", fi=128), [128, FO, 128], bf16, "wq_sb")
            wk_sb = ldconst(wk.rearrange("(fo fi) o -> fi fo o", fi=128), [128, FO, 128], bf16, "wk_sb")
            wv_sb = ldconst(wv.rearrange("(fo fi) o -> fi fo o", fi=128), [128, FO, 128], bf16, "wv_sb")
            cq_sb = ldconst(cqw.rearrange("(fo fi) o -> fi fo o", fi=128), [128, FO, 128], bf16, "cq_sb")
            ck_sb = ldconst(ckw.rearrange("(fo fi) o -> fi fo o", fi=128), [128, FO, 128], bf16, "ck_sb")
            cv_sb = ldconst(cvw.rearrange("(fo fi) o -> fi fo o", fi=128), [128, FO, 128], bf16, "cv_sb")
            psel_sb = ldconst(psel[:], [128, FO, 128], f32, "psel_sb")

            bq_sb = ldconst(bq[:, None], [128, 1], f32, "bq_sb")
            bk_sb = ldconst(bk[:, None], [128, 1], f32, "bk_sb")
            bv_sb = ldconst(bv[:, None], [128, 1], f32, "bv_sb")
            cqb_sb = ldconst(cqb[:, None], [128, 1], f32, "cqb_sb")
            ckb_sb = ldconst(ckb[:, None], [128, 1], f32, "ckb_sb")
            cvb_sb = ldconst(cvb[:, None], [128, 1], f32, "cvb_sb")

            g1 = ldconst(ln1g[:, :, None], [128, FO, 1], f32, "g1")
            be1 = ldconst(ln1b[:, :, None], [128, FO, 1], f32, "be1")
            g2 = ldconst(ln2g[:, :, None], [128, FO, 1], f32, "g2")
            be2 = ldconst(ln2b[:, :, None], [128, FO, 1], f32, "be2")
            g3 = ldconst(ln3g[:, :, None], [128, FO, 1], f32, "g3")
            be3 = ldconst(ln3b[:, :, None], [128, FO, 1], f32, "be3")

            # bias rows broadcast to all 128 partitions at DMA time (step-0 src)
            def bcast_vec(ap_, n, name):
                full = const.tile([128, n], f32, tag=name)
                nc.sync.dma_start(full[:], ap_[None, :].to_broadcast((128, n)))
                return full

            saob_bc = bcast_vec(saob, D, "saob_bc")
            cob_bc = bcast_vec(cob, D, "cob_bc")
            b2_bc = bcast_vec(b2, D, "b2_bc")
            b1_bc = bcast_vec(b1, DFF, "b1_bc")

            dram_x = dram.tile([T, D], f32)
            dram_x3 = dram.tile([T, D], f32)

            def ln_to_dram(src_dram, out_dram, ntt):
                """LayerNorm (normalize only): src_dram [ntt*128, D] f32 -> bf16 out_dram."""
                for tt in range(ntt):
                    xt = sb.tile([128, 1024], f32, tag="ln_in", name=f"ln_in_{tt}")
                    nc.sync.dma_start(xt[:], src_dram[tt * 128:(tt + 1) * 128, :])
                    stats = sb.tile([128, 2, 6], f32, tag="ln_stats", name=f"ln_st_{tt}")
                    nc.vector.bn_stats(stats[:, 0, :], xt[:, 0:512])
                    nc.vector.bn_stats(stats[:, 1, :], xt[:, 512:1024])
                    mv = sb.tile([128, 2], f32, tag="ln_mv", name=f"ln_mv_{tt}")
                    nc.vector.bn_aggr(mv[:], stats[:])
                    eps = sb.tile([128, 1], f32, tag="ln_eps", name=f"ln_eps_{tt}")
                    nc.vector.memset(eps[:], EPS)
                    rstd = sb.tile([128, 1], f32, tag="ln_rstd", name=f"ln_rs_{tt}")
                    nc.scalar.activation(rstd[:], mv[:, 1:2], AF.Sqrt, bias=eps[:], scale=1.0)
                    nc.vector.reciprocal(rstd[:], rstd[:])
                    xn = sb.tile([128, 1024], bf16, tag="ln_xn", name=f"ln_xn_{tt}")
                    nc.vector.tensor_scalar(xn[:], xt[:], scalar1=mv[:, 0:1], scalar2=rstd[:],
                                            op0=ALU.subtract, op1=ALU.mult)
                    nc.sync.dma_start(out_dram[tt * 128:(tt + 1) * 128, :], xn[:])

            def transpose_load(dst, src_dram, g_sb, b_sb):
                for fo in range(FO):
                    nc.sync.dma_start_transpose(dst[:, fo, :],
                                                src_dram[:, fo * 128:(fo + 1) * 128])
                    nc.vector.tensor_scalar(dst[:, fo, :], dst[:, fo, :],
                                            scalar1=g_sb[:, fo, :], scalar2=b_sb[:, fo, :],
                                            op0=ALU.mult, op1=ALU.add)

            # ---- P1: LN1(queries) -> qn_T ----
            qn_dram = dram.tile([T, D], bf16)
            ln_to_dram(queries, qn_dram, FO)
            if DBG:
                nc.sync.dma_start(dbg["dbg_qn"][:], qn_dram[:])
            qn_T = fm.tile([128, FO, T], bf16, tag="fmT", name="qn_T")
            transpose_load(qn_T, qn_dram, g1, be1)

            # ---- P2: self-attn qkv (2 heads, feature-major) ----
            qs_T = big.tile([128, T], bf16, tag="qs_T")
            ks_T = big.tile([128, T], bf16, tag="ks_T")
            vs_T = big.tile([128, T], bf16, tag="vs_T")
            for wi, (w_sb, bias_sb, dst) in enumerate(((wq_sb, bq_sb, qs_T), (wk_sb, bk_sb, ks_T),
                                                      (wv_sb, None, vs_T))):
                for tc2 in range(2):
                    ps = psA.tile([128, 512], f32, tag="ps512", name=f"ps_qkv{wi}_{tc2}")
                    for f in range(FO):
                        nc.tensor.matmul(ps[:], w_sb[:, f, :], qn_T[:, f, tc2 * 512:(tc2 + 1) * 512],
                                         start=(f == 0), stop=(f == FO - 1))
                    if bias_sb is not None:
                        nc.scalar.activation(dst[:, tc2 * 512:(tc2 + 1) * 512], ps[:],
                                             AF.Identity, bias=bias_sb[:])
                    else:
                        nc.scalar.activation(dst[:, tc2 * 512:(tc2 + 1) * 512], ps[:], AF.Copy)
            vaug_s = big.tile([128, B, 2, HPC, 128], bf16, tag="vaug_s")
            nc.vector.memset(vaug_s[:, :, :, :, 64:65], 1.0)
            for b in range(B):
                for kt in range(2):
                    for hh in range(HPC):
                        nc.sync.dma_start_transpose(
                            vaug_s[:, b, kt, hh, 0:64],
                            vs_T[hh * 64:(hh + 1) * 64, b * 256 + kt * 128: b * 256 + (kt + 1) * 128])

            # ---- self-attn + AG#1 ----
            o_self = big.tile([128, T], bf16, tag="o_self")
            for b in range(B):
                for hh in range(HPC):
                    pso = psO.tile([65, 256], f32, tag="pso", name=f"psoS_{b}_{hh}")
                    for kt in range(2):
                        pss = psS.tile([128, 256], f32, tag="pss", name=f"pssS_{b}_{hh}_{kt}")
                        nc.tensor.matmul(
                            pss[:],
                            ks_T[hh * 64:(hh + 1) * 64, b * 256 + kt * 128: b * 256 + (kt + 1) * 128],
                            qs_T[hh * 64:(hh + 1) * 64, b * 256:(b + 1) * 256],
                            start=True, stop=True)
                        pT = sb.tile([128, 256], bf16, tag="pT", name=f"pTS_{b}_{hh}_{kt}")
                        nc.scalar.activation(pT[:], pss[:], AF.Exp, scale=0.125)
                        nc.tensor.matmul(pso[:], vaug_s[:, b, kt, hh, 0:65], pT[:],
                                         start=(kt == 0), stop=(kt == 1))
                    rinv = sb.tile([1, 256], f32, tag="rinv", name=f"riS_{b}_{hh}")
                    nc.vector.reciprocal(rinv[:], pso[64:65, :])
                    rb = sb.tile([64, 256], f32, tag="rb", name=f"rbS_{b}_{hh}")
                    nc.gpsimd.partition_broadcast(rb[:], rinv[:])
                    oslice = o_self[hh * 64:(hh + 1) * 64, b * 256:(b + 1) * 256]
                    nc.vector.tensor_tensor(oslice, pso[0:64, :], rb[:], ALU.mult)
                    nc.vector.tensor_scalar_add(oslice, oslice, bv_sb[hh * 64:(hh + 1) * 64, :])
            if DBG:
                nc.sync.dma_start(dbg["dbg_qs"][:], qs_T[:])
                nc.sync.dma_start(dbg["dbg_ks"][:], ks_T[:])
                nc.sync.dma_start(dbg["dbg_vs"][:], vs_T[:])
                nc.sync.dma_start(dbg["dbg_oself"][:], o_self[:])
            ag1_in = dram.tile([128, T], bf16)
            ag1_out = dram.tile([NC, 128, T], bf16)
            nc.sync.dma_start(ag1_in[:], o_self[:])
            nc.gpsimd.collective_compute(
                "AllGather", ALU.bypass, replica_groups=[list(range(NC))],
                ins=[ag1_in[:].opt()], outs=[ag1_out[:].opt()])
            o_full = fm.tile([128, FO, T], bf16, tag="fmT", name="o_full")
            for s in range(NC):
                nc.sync.dma_start(o_full[:, s, :], ag1_out[s])

            # ---- P3: sa_out projection (replicated) + residual -> x (DRAM) ----
            for oc in range(2):
                saow_c = wstream.tile([128, FO, 512], bf16, tag="wc", name=f"saow_c{oc}")
                nc.sync.dma_start(saow_c[:], saow_r[:, :, oc * 512:(oc + 1) * 512])
                for tt in range(FO):
                    ps = psA.tile([128, 512], f32, tag="ps512", name=f"ps_x{oc}_{tt}")
                    for s in range(FO):
                        nc.tensor.matmul(ps[:], o_full[:, s, tt * 128:(tt + 1) * 128],
                                         saow_c[:, s, :],
                                         start=(s == 0), stop=(s == FO - 1))
                    qres = rstream.tile([128, 512], f32, tag="qres", name=f"qres{oc}_{tt}")
                    nc.sync.dma_start(qres[:], queries[tt * 128:(tt + 1) * 128, oc * 512:(oc + 1) * 512])
                    xs = rstream.tile([128, 512], f32, tag="xs", name=f"xs{oc}_{tt}")
                    nc.vector.tensor_tensor(xs[:], ps[:], qres[:], ALU.add)
                    nc.vector.tensor_tensor(xs[:], xs[:], saob_bc[:, oc * 512:(oc + 1) * 512], ALU.add)
                    nc.sync.dma_start(dram_x[tt * 128:(tt + 1) * 128, oc * 512:(oc + 1) * 512], xs[:])

            if DBG:
                nc.sync.dma_start(dbg["dbg_x"][:], dram_x[:])
                nc.sync.dma_start(dbg["dbg_ag1"][:], ag1_out[:])
            # ---- P4/P5: LN2(x) -> xn2_T -> cross q ----
            xn2_dram = dram.tile([T, D], bf16)
            ln_to_dram(dram_x, xn2_dram, FO)
            xn2_T = fm.tile([128, FO, T], bf16, tag="fmT", name="xn2_T")
            transpose_load(xn2_T, xn2_dram, g2, be2)
            q2_T = big.tile([128, T], bf16, tag="q2_T")
            for tc2 in range(2):
                ps = psA.tile([128, 512], f32, tag="ps512", name=f"ps_q2{tc2}")
                for f in range(FO):
                    nc.tensor.matmul(ps[:], cq_sb[:, f, :], xn2_T[:, f, tc2 * 512:(tc2 + 1) * 512],
                                     start=(f == 0), stop=(f == FO - 1))
                nc.scalar.activation(q2_T[:, tc2 * 512:(tc2 + 1) * 512], ps[:],
                                     AF.Identity, bias=cqb_sb[:])

            if DBG:
                nc.sync.dma_start(dbg["dbg_q2"][:], q2_T[:])
            # ---- P6: cross-attention over the full context ----
            o_cross = big.tile([128, T], bf16, tag="o_cross")
            vaug_slots = []
            for i in range(2):
                vsl = big.tile([128, 4, HPC, 128], bf16, tag=f"vaugsl{i}", name=f"vaugsl{i}")
                nc.vector.memset(vsl[:, :, :, 64:65], 1.0)
                vaug_slots.append(vsl)
            for b in range(B):
                pso_h = [psO.tile([65, 256], f32, tag="pso", name=f"psoC_{b}_{i}")
                         for i in range(HPC)]
                for sc in range(NSC):
                    base = b * S + sc * SCH
                    ctx_T = stream.tile([128, FO, SCH], bf16, tag="ctx_T", name=f"ctxT_{b}_{sc}")
                    nc.sync.dma_start(ctx_T[:], ctx_r[:, :, base:base + SCH])
                    psk = psA.tile([128, SCH], f32, tag="ps512", name=f"ps_k{b}_{sc}")
                    for f in range(FO):
                        nc.tensor.matmul(psk[:], ck_sb[:, f, :], ctx_T[:, f, :],
                                         start=(f == 0), stop=(f == FO - 1))
                    kc = stream.tile([128, SCH], bf16, tag="kc", name=f"kc{b}_{sc}")
                    nc.scalar.activation(kc[:], psk[:], AF.Identity, bias=ckb_sb[:])
                    # V chunk token-major: four 128-token tiles in one PSUM bank
                    psv = psA.tile([128, SCH], f32, tag="ps512", name=f"ps_v{b}_{sc}")
                    for kt in range(4):
                        for f in range(FO):
                            nc.tensor.matmul(psv[:, kt * 128:(kt + 1) * 128],
                                             ctx_T[:, f, kt * 128:(kt + 1) * 128], cv_sb[:, f, :],
                                             start=(f == 0), stop=(f == FO - 1))
                    vaug = vaug_slots[sc % 2]
                    for kt in range(4):
                        for hh in range(HPC):
                            nc.vector.tensor_copy(
                                vaug[:, kt, hh, 0:64],
                                psv[:, kt * 128 + hh * 64: kt * 128 + hh * 64 + 64])
                    for hh in range(HPC):
                        for kt in range(4):
                            pss = psS.tile([128, 256], f32, tag="pss", name=f"pssC_{b}_{sc}_{hh}_{kt}")
                            nc.tensor.matmul(
                                pss[:], kc[hh * 64:(hh + 1) * 64, kt * 128:(kt + 1) * 128],
                                q2_T[hh * 64:(hh + 1) * 64, b * 256:(b + 1) * 256],
                                start=True, stop=True)
                            pT = sb.tile([128, 256], bf16, tag="pT", name=f"pTC_{b}_{sc}_{hh}_{kt}")
                            nc.scalar.activation(pT[:], pss[:], AF.Exp, scale=0.125)
                            nc.tensor.matmul(pso_h[hh][:], vaug[:, kt, hh, 0:65], pT[:],
                                             start=(sc == 0 and kt == 0),
                                             stop=(sc == NSC - 1 and kt == 3))
                for hh in range(HPC):
                    rinv = sb.tile([1, 256], f32, tag="rinv", name=f"riC_{b}_{hh}")
                    nc.vector.reciprocal(rinv[:], pso_h[hh][64:65, :])
                    rb = sb.tile([64, 256], f32, tag="rb", name=f"rbC_{b}_{hh}")
                    nc.gpsimd.partition_broadcast(rb[:], rinv[:])
                    oslice = o_cross[hh * 64:(hh + 1) * 64, b * 256:(b + 1) * 256]
                    nc.vector.tensor_tensor(oslice, pso_h[hh][0:64, :], rb[:], ALU.mult)
                    nc.vector.tensor_scalar_add(oslice, oslice, cvb_sb[hh * 64:(hh + 1) * 64, :])

            if DBG:
                nc.sync.dma_start(dbg["dbg_ocross"][:], o_cross[:])
            ag2_in = dram.tile([128, T], bf16)
            ag2_out = dram.tile([NC, 128, T], bf16)
            nc.sync.dma_start(ag2_in[:], o_cross[:])
            nc.gpsimd.collective_compute(
                "AllGather", ALU.bypass, replica_groups=[list(range(NC))],
                ins=[ag2_in[:].opt()], outs=[ag2_out[:].opt()])
            oc_full = fm.tile([128, FO, T], bf16, tag="fmT", name="oc_full")
            for s in range(NC):
                nc.sync.dma_start(oc_full[:, s, :], ag2_out[s])

            # ---- P7: co projection (replicated) + residual -> x3 (DRAM) ----
            for oc in range(2):
                cow_c = wstream.tile([128, FO, 512], bf16, tag="wc", name=f"cow_c{oc}")
                nc.sync.dma_start(cow_c[:], cow_r[:, :, oc * 512:(oc + 1) * 512])
                for tt in range(FO):
                    ps = psA.tile([128, 512], f32, tag="ps512", name=f"ps_x3{oc}_{tt}")
                    for s in range(FO):
                        nc.tensor.matmul(ps[:], oc_full[:, s, tt * 128:(tt + 1) * 128],
                                         cow_c[:, s, :],
                                         start=(s == 0), stop=(s == FO - 1))
                    xres = rstream.tile([128, 512], f32, tag="qres", name=f"xres{oc}_{tt}")
                    nc.sync.dma_start(xres[:], dram_x[tt * 128:(tt + 1) * 128, oc * 512:(oc + 1) * 512])
                    xs = rstream.tile([128, 512], f32, tag="xs", name=f"xs3{oc}_{tt}")
                    nc.vector.tensor_tensor(xs[:], ps[:], xres[:], ALU.add)
                    nc.vector.tensor_tensor(xs[:], xs[:], cob_bc[:, oc * 512:(oc + 1) * 512], ALU.add)
                    nc.sync.dma_start(dram_x3[tt * 128:(tt + 1) * 128, oc * 512:(oc + 1) * 512], xs[:])

            if DBG:
                nc.sync.dma_start(dbg["dbg_x3"][:], dram_x3[:])
            # ---- P8: select own 128 tokens (one-hot matmul, fp32 exact) ----
            x3_own = big.tile([128, D], f32, tag="x3_own")
            for oc in range(2):
                ps = psA.tile([128, 512], f32, tag="ps512", name=f"ps_sel{oc}")
                for tt in range(FO):
                    rt = rstream.tile([128, 512], f32, tag="qres", name=f"x3t{oc}_{tt}")
                    nc.sync.dma_start(rt[:], dram_x3[tt * 128:(tt + 1) * 128, oc * 512:(oc + 1) * 512])
                    nc.tensor.matmul(ps[:], psel_sb[:, tt, :], rt[:],
                                     start=(tt == 0), stop=(tt == FO - 1))
                nc.vector.tensor_copy(x3_own[:, oc * 512:(oc + 1) * 512], ps[:])

            if DBG:
                nc.sync.dma_start(dbg["dbg_x3own"][:], x3_own[:])
            # ---- P9: LN3 on own tokens -> xn3_T ----
            x3o_dram = dram.tile([128, D], f32)
            nc.sync.dma_start(x3o_dram[:], x3_own[:])
            xn3_dram = dram.tile([128, D], bf16)
            ln_to_dram(x3o_dram, xn3_dram, 1)
            xn3_T = big.tile([128, FO, 128], bf16, tag="xn3_T")
            for fo in range(FO):
                nc.sync.dma_start_transpose(xn3_T[:, fo, :], xn3_dram[:, fo * 128:(fo + 1) * 128])
                nc.vector.tensor_scalar(xn3_T[:, fo, :], xn3_T[:, fo, :],
                                        scalar1=g3[:, fo, :], scalar2=be3[:, fo, :],
                                        op0=ALU.mult, op1=ALU.add)

            # ---- P10: FFN on own tokens ----
            h_sb = big.tile([128, DFF], bf16, tag="h_sb")
            for dc in range(8):
                w1c = wstream.tile([128, FO, 512], bf16, tag="wc", name=f"w1c{dc}")
                nc.sync.dma_start(w1c[:], w1_r[:, :, dc * 512:(dc + 1) * 512])
                psh = psA.tile([128, 512], f32, tag="ps512", name=f"ps_h{dc}")
                for f in range(FO):
                    nc.tensor.matmul(psh[:], xn3_T[:, f, :], w1c[:, f, :],
                                     start=(f == 0), stop=(f == FO - 1))
                nc.vector.tensor_tensor(psh[:], psh[:], b1_bc[:, dc * 512:(dc + 1) * 512], ALU.add)
                nc.scalar.activation(h_sb[:, dc * 512:(dc + 1) * 512], psh[:], AF.Gelu)
            if DBG:
                nc.sync.dma_start(dbg["dbg_h"][:], h_sb[:])
            h_T = big.tile([128, 32, 128], bf16, tag="h_T")
            for dt in range(32):
                nc.sync.dma_start_transpose(h_T[:, dt, :], h_sb[:, dt * 128:(dt + 1) * 128])
            out_sb = big.tile([128, D], f32, tag="out_sb")
            for oc in range(2):
                psy = psA.tile([128, 512], f32, tag="ps512", name=f"ps_y{oc}")
                for dg in range(4):
                    w2c = wstream.tile([128, FO, 512], bf16, tag="wc", name=f"w2c{oc}_{dg}")
                    nc.sync.dma_start(w2c[:], w2_r[:, dg * FO:(dg + 1) * FO, oc * 512:(oc + 1) * 512])
                    for j in range(FO):
                        dt = dg * FO + j
                        nc.tensor.matmul(psy[:], h_T[:, dt, :], w2c[:, j, :],
                                         start=(dt == 0), stop=(dt == 31))
                ys = out_sb[:, oc * 512:(oc + 1) * 512]
                nc.vector.tensor_tensor(ys, psy[:], x3_own[:, oc * 512:(oc + 1) * 512], ALU.add)
                nc.vector.tensor_tensor(ys, ys, b2_bc[:, oc * 512:(oc + 1) * 512], ALU.add)
            nc.sync.dma_start(out[:], out_sb[:])

    nc.compile()
    return nc


def _pack_ln(v):
    return np.ascontiguousarray(np.asarray(v, dtype=np.float32).reshape(FO, 128).T)


def _get_nc():
    if "nc" not in _CACHE:
        _CACHE["nc"] = _build()
    return _CACHE["nc"]


def kernel(**inputs):
    nc = _get_nc()
    inp = {k: np.asarray(v) for k, v in inputs.items()}

    def bf(a):
        return np.ascontiguousarray(a).astype(BF16NP)

    queries = np.ascontiguousarray(inp["queries"].reshape(T, D).astype(np.float32))
    ctx = bf(inp["context"].reshape(B * S, D).T)
    sa_in_w = inp["sa_in_w"]; sa_in_b = inp["sa_in_b"]
    shared = {
        "queries": queries, "ctx": ctx,
        "ln1g": _pack_ln(inp["ln1_g"]), "ln1b": _pack_ln(inp["ln1_b"]),
        "ln2g": _pack_ln(inp["ln2_g"]), "ln2b": _pack_ln(inp["ln2_b"]),
        "ln3g": _pack_ln(inp["ln3_g"]), "ln3b": _pack_ln(inp["ln3_b"]),
        "saow": bf(inp["sa_out_w"].T), "saob": np.asarray(inp["sa_out_b"], np.float32),
        "cow": bf(inp["co_w"].T), "cob": np.asarray(inp["co_b"], np.float32),
        "w1": bf(inp["w1"].T), "b1": np.asarray(inp["b1"], np.float32),
        "w2": bf(inp["w2"].T), "b2": np.asarray(inp["b2"], np.float32),
    }
    in_maps = []
    eye = np.eye(128, dtype=np.float32)
    for c in range(NC):
        r = slice(c * 128, (c + 1) * 128)
        psel = np.zeros((128, FO, 128), np.float32)
        psel[:, c, :] = eye
        m = dict(shared)
        m.update({
            "wq": bf(sa_in_w[0 * D:1 * D][r].T), "bq": np.asarray(sa_in_b[0 * D:1 * D][r], np.float32),
            "wk": bf(sa_in_w[1 * D:2 * D][r].T), "bk": np.asarray(sa_in_b[1 * D:2 * D][r], np.float32),
            "wv": bf(sa_in_w[2 * D:3 * D][r].T), "bv": np.asarray(sa_in_b[2 * D:3 * D][r], np.float32),
            "cqw": bf(inp["cq_w"][r].T), "cqb": np.asarray(inp["cq_b"][r], np.float32),
            "ckw": bf(inp["ck_w"][r].T), "ckb": np.asarray(inp["ck_b"][r], np.float32),
            "cvw": bf(inp["cv_w"][r].T), "cvb": np.asarray(inp["cv_b"][r], np.float32),
            "psel": psel,
        })
        in_maps.append(m)

    res = run_bass_kernel_spmd(nc, in_maps, core_ids=list(range(NC)),
                               **_CACHE.get("run_kwargs", {}))
    _CACHE["last_result"] = res
    out = np.concatenate([np.asarray(res.results[c]["out"]) for c in range(NC)], axis=0)
    return out.reshape(B, Q, D).astype(np.float32)



# revision 6
# speedup vs baseline: 1.2658x; 1.2658x over previous
"""CompressionTransformerLayer on 8 TRN2 NeuronCores (Bass/Tile), v2.

Sharding: tensor-parallel by heads (16 heads -> 2 per core), context replicated.
Key structure vs v1:
 - Cross-attn K/V projection decoupled from the q2 critical path: ctx streamed
   fp8 (32MB/core), K/V projected with fp8 DoubleRow matmuls into resident
   SBUF arrays (kfull fp8 feature-major, vaug fp8 token-major w/ ones column).
   These have no deps on the self-attn chain, so the Tile scheduler overlaps
   them with LN1/self-attn/AG1/sa_out/LN2/q2.
 - Scores: fp8 (K scaled x16 host-side, folded into exp scale), two heads
   row-packed via base_partition for PE tile concurrency, 4 score MMs batched
   into one [128,1024] 2-bank PSUM tile -> single EXP (amortizes ScalarE's
   352-cycle fixed cost) -> pT fp8.
 - attn@V: DoubleRow over group-pairs (2x128 tokens per MM).
 - Per-batch AllGather of cross-attn outputs so co-projection + residual
   overlap the next batch's score stream.
 - x / x3 residuals resident in SBUF (bf16), no DRAM bounce; x3 built in-place.
 - FFN: h computed dff-major (no h transposes), gelu+bias fused on ScalarE.
All matmul accumulation fp32 in PSUM; LN/softmax-normalize in fp32.
"""
import sys
sys.path.insert(0, "/opt/trn_rl_repo")
sys.path.insert(0, "/root/.axon_site")

import contextlib
import numpy as np

import concourse.bass as bass
import concourse.mybir as mybir
import concourse.tile as tile
from concourse import bacc
from concourse.bass_utils import run_bass_kernel_spmd

f32, bf16, fp8 = mybir.dt.float32, mybir.dt.bfloat16, mybir.dt.float8e4
AF = mybir.ActivationFunctionType
ALU = mybir.AluOpType
DR = mybir.MatmulPerfMode.DoubleRow
BF16NP = mybir.dt.np(bf16)
FP8NP = mybir.dt.np(fp8)

D, H, HD, DFF = 1024, 16, 64, 4096
B, Q, S = 4, 256, 8192
NC = 8
T = B * Q            # 1024 flattened query tokens
HPC = H // NC        # 2 heads per core
FO = D // 128        # 8 feature tiles
SCH = 512            # context chunk (tokens)
NSC = S // SCH       # 16 chunks per batch element
NCH = B * NSC        # 64 chunks total
GPB = 32             # group-pairs (of 2x128 tokens) per batch
KSC = 16.0           # host-side scale on ck/cv weights (fp8 subnormal avoidance)
EPS = 1e-5

_CACHE = {}


def _build():
    nc = bacc.Bacc("TRN2", target_bir_lowering=False, debug=False,
                   enable_asserts=True, num_devices=NC)

    def din(name, shape, dt=bf16):
        return nc.dram_tensor(name, shape, dt, kind="ExternalInput").ap()

    queries = din("queries", [T, D], f32)
    ctx = din("ctx", [D, B * S], fp8)
    ln1g = din("ln1g", [128, FO], f32); ln1b = din("ln1b", [128, FO], f32)
    ln2g = din("ln2g", [128, FO], f32); ln2b = din("ln2b", [128, FO], f32)
    ln3g = din("ln3g", [128, FO], f32); ln3b = din("ln3b", [128, FO], f32)
    wq = din("wq", [D, 128]); wk = din("wk", [D, 128]); wv = din("wv", [D, 128])
    bq = din("bq", [128], f32); bk = din("bk", [128], f32); bv = din("bv", [128], f32)
    saow = din("saow", [D, D]); saob = din("saob", [D], bf16)
    cqw = din("cqw", [D, 128]); cqb = din("cqb", [128], f32)
    ckw = din("ckw", [D, 128], fp8); ckb = din("ckb", [128], f32)
    cvw = din("cvw", [D, 128], fp8); cvb = din("cvb", [128], f32)
    cow = din("cow", [D, D]); cob = din("cob", [D], bf16)
    w1 = din("w1", [D, DFF]); b1 = din("b1", [128, 32], f32)
    w2 = din("w2", [DFF, D]); b2 = din("b2", [D], f32)
    psel = din("psel", [128, FO, 128])
    ident_d = din("ident", [128, 128])

    out = nc.dram_tensor("out", [128, D], f32, kind="ExternalOutput").ap()

    ctx_r = ctx.rearrange("(fo fi) t -> fi fo t", fi=128)
    w1_r = w1.rearrange("(fo fi) n -> fi fo n", fi=128)
    w2_r = w2.rearrange("(dg fi) n -> fi dg n", fi=128)
    saow_r = saow.rearrange("(fo fi) n -> fi fo n", fi=128)
    cow_r = cow.rearrange("(fo fi) n -> fi fo n", fi=128)

    with tile.TileContext(nc) as tc:
        with contextlib.ExitStack() as ctxs:
            const = ctxs.enter_context(tc.tile_pool(name="const", bufs=1))
            big = ctxs.enter_context(tc.tile_pool(name="big", bufs=1))
            fm = ctxs.enter_context(tc.tile_pool(name="fm", bufs=1))
            sb = ctxs.enter_context(tc.tile_pool(name="sb", bufs=2))
            stream = ctxs.enter_context(tc.tile_pool(name="stream", bufs=2))
            pts = ctxs.enter_context(tc.tile_pool(name="pts", bufs=3))
            rstream = ctxs.enter_context(tc.tile_pool(name="rstream", bufs=2))
            wstream = ctxs.enter_context(tc.tile_pool(name="wstream", bufs=2))
            dram = ctxs.enter_context(tc.tile_pool(name="dram", bufs=1, space="DRAM"))
            psA = ctxs.enter_context(tc.tile_pool(name="psA", bufs=2, space="PSUM"))
            psS = ctxs.enter_context(tc.tile_pool(name="psS", bufs=2, space="PSUM"))
            psX = ctxs.enter_context(tc.tile_pool(name="psX", bufs=2, space="PSUM"))
            psO = ctxs.enter_context(tc.tile_pool(name="psO", bufs=2, space="PSUM"))

            def ldconst(ap_, shape, dt, name):
                t = const.tile(shape, dt, tag=name)
                nc.sync.dma_start(t[:], ap_)
                return t

            wq_sb = ldconst(wq.rearrange("(fo fi) o -> fi fo o", fi=128), [128, FO, 128], bf16, "wq_sb")
            wk_sb = ldconst(wk.rearrange("(fo fi) o -> fi fo o", fi=128), [128, FO, 128], bf16, "wk_sb")
            wv_sb = ldconst(wv.rearrange("(fo fi) o -> fi fo o", fi=128), [128, FO, 128], bf16, "wv_sb")
            cq_sb = ldconst(cqw.rearrange("(fo fi) o -> fi fo o", fi=128), [128, FO, 128], bf16, "cq_sb")
            ck_sb = ldconst(ckw.rearrange("(fo fi) o -> fi fo o", fi=128), [128, FO, 128], fp8, "ck_sb")
            cv_sb = ldconst(cvw.rearrange("(fo fi) o -> fi fo o", fi=128), [128, FO, 128], fp8, "cv_sb")
            psel_sb = ldconst(psel[:], [128, FO, 128], bf16, "psel_sb")
            ident = ldconst(ident_d[:], [128, 128], bf16, "ident")

            bq_sb = ldconst(bq[:, None], [128, 1], f32, "bq_sb")
            bk_sb = ldconst(bk[:, None], [128, 1], f32, "bk_sb")
            bv_sb = ldconst(bv[:, None], [128, 1], f32, "bv_sb")
            cqb_sb = ldconst(cqb[:, None], [128, 1], f32, "cqb_sb")
            ckb_sb = ldconst(ckb[:, None], [128, 1], f32, "ckb_sb")
            cvb_sb = ldconst(cvb[:, None], [128, 1], f32, "cvb_sb")
            b1_sb = ldconst(b1[:], [128, 32], f32, "b1_sb")

            g1 = ldconst(ln1g[:, :, None], [128, FO, 1], f32, "g1")
            be1 = ldconst(ln1b[:, :, None], [128, FO, 1], f32, "be1")
            g2 = ldconst(ln2g[:, :, None], [128, FO, 1], f32, "g2")
            be2 = ldconst(ln2b[:, :, None], [128, FO, 1], f32, "be2")
            g3 = ldconst(ln3g[:, :, None], [128, FO, 1], f32, "g3")
            be3 = ldconst(ln3b[:, :, None], [128, FO, 1], f32, "be3")

            def bcast_vec(ap_, n, dt, name):
                full = const.tile([128, n], dt, tag=name)
                nc.sync.dma_start(full[:], ap_[None, :].to_broadcast((128, n)))
                return full

            saob_bc = bcast_vec(saob, D, bf16, "saob_bc")
            cob_bc = bcast_vec(cob, D, bf16, "cob_bc")
            b2_bc = bcast_vec(b2, D, f32, "b2_bc")

            # ---- persistent SBUF state ----
            kfull = big.tile([128, B * S], fp8, tag="kfull")       # 32KB/part
            vaug = big.tile([128, HPC, 256, 80], fp8, tag="vaug")  # 40KB/part
            x_sb = big.tile([128, FO, D], bf16, tag="x_sb")        # x, then x3 in place
            q2_sb = big.tile([128, T], fp8, tag="q2_sb")
            nc.vector.memset(vaug[:, :, :, 64:80], 0.0)
            nc.vector.memset(vaug[:, :, :, 64:65], 1.0)

            def ln_tiles(src_getter, out_cb, ntt, pref):
                """LayerNorm (normalize only) over [128,1024] f32/bf16 tiles."""
                for tt in range(ntt):
                    xt = src_getter(tt)
                    stats = sb.tile([128, 2, 6], f32, tag="ln_stats", name=f"{pref}_st{tt}")
                    nc.vector.bn_stats(stats[:, 0, :], xt[:, 0:512])
                    nc.vector.bn_stats(stats[:, 1, :], xt[:, 512:1024])
                    mv = sb.tile([128, 2], f32, tag="ln_mv", name=f"{pref}_mv{tt}")
                    nc.vector.bn_aggr(mv[:], stats[:])
                    eps = sb.tile([128, 1], f32, tag="ln_eps", name=f"{pref}_eps{tt}")
                    nc.vector.memset(eps[:], EPS)
                    rstd = sb.tile([128, 1], f32, tag="ln_rstd", name=f"{pref}_rs{tt}")
                    nc.scalar.activation(rstd[:], mv[:, 1:2], AF.Sqrt, bias=eps[:], scale=1.0)
                    nc.vector.reciprocal(rstd[:], rstd[:])
                    xn = sb.tile([128, 1024], bf16, tag="ln_xn", name=f"{pref}_xn{tt}")
                    nc.vector.tensor_scalar(xn[:], xt[:], scalar1=mv[:, 0:1], scalar2=rstd[:],
                                            op0=ALU.subtract, op1=ALU.mult)
                    out_cb(tt, xn)

            # ---- P1: LN1(queries) -> qn_dram -> qn_T (feature-major) ----
            qn_dram = dram.tile([T, D], bf16)

            def ln1_src(tt):
                xt = sb.tile([128, 1024], f32, tag="ln_in", name=f"ln1_in{tt}")
                nc.sync.dma_start(xt[:], queries[tt * 128:(tt + 1) * 128, :])
                return xt

            ln_tiles(ln1_src, lambda tt, xn: nc.sync.dma_start(
                qn_dram[tt * 128:(tt + 1) * 128, :], xn[:]), FO, "ln1")

            qn_T = fm.tile([128, FO, T], bf16, tag="fmT", name="qn_T")
            for fo in range(FO):
                nc.sync.dma_start_transpose(qn_T[:, fo, :], qn_dram[:, fo * 128:(fo + 1) * 128])
                nc.vector.tensor_scalar(qn_T[:, fo, :], qn_T[:, fo, :],
                                        scalar1=g1[:, fo, :], scalar2=be1[:, fo, :],
                                        op0=ALU.mult, op1=ALU.add)

            # ---- P2: self-attn qkv (2 heads, feature-major) ----
            qs_T = big.tile([128, T], bf16, tag="qs_T")
            ks_T = big.tile([128, T], bf16, tag="ks_T")
            vs_T = big.tile([128, T], bf16, tag="vs_T")
            for wi, (w_sb, bias_sb, dst) in enumerate(((wq_sb, bq_sb, qs_T), (wk_sb, bk_sb, ks_T),
                                                      (wv_sb, None, vs_T))):
                for tc2 in range(2):
                    ps = psA.tile([128, 512], f32, tag="ps512", name=f"ps_qkv{wi}_{tc2}")
                    for f in range(FO):
                        nc.tensor.matmul(ps[:], w_sb[:, f, :], qn_T[:, f, tc2 * 512:(tc2 + 1) * 512],
                                         start=(f == 0), stop=(f == FO - 1))
                    if bias_sb is not None:
                        nc.scalar.activation(dst[:, tc2 * 512:(tc2 + 1) * 512], ps[:],
                                             AF.Identity, bias=bias_sb[:])
                    else:
                        nc.scalar.activation(dst[:, tc2 * 512:(tc2 + 1) * 512], ps[:], AF.Copy)

            # self V -> token-major via PE transpose
            vaug_s = big.tile([128, B, 2, HPC, 66], bf16, tag="vaug_s")
            nc.vector.memset(vaug_s[:, :, :, :, 64:66], 0.0)
            nc.vector.memset(vaug_s[:, :, :, :, 64:65], 1.0)
            for b in range(B):
                for kt in range(2):
                    ptr = psA.tile([128, 128], bf16, tag="ps512", name=f"ptrS_{b}_{kt}")
                    nc.tensor.transpose(
                        ptr[:], vs_T[:, b * 256 + kt * 128: b * 256 + (kt + 1) * 128], ident[:])
                    for hh in range(HPC):
                        nc.vector.tensor_copy(vaug_s[:, b, kt, hh, 0:64],
                                              ptr[:, hh * 64:(hh + 1) * 64])

            # ---- self-attn scores/exp/AV + normalize -> o_self ----
            o_self = big.tile([128, T], bf16, tag="o_self")
            for b in range(B):
                for hh in range(HPC):
                    pss = psX.tile([128, 2, 256], f32, tag="psx", name=f"pssS_{b}_{hh}")
                    for kt in range(2):
                        nc.tensor.matmul(
                            pss[:, kt, :],
                            ks_T[hh * 64:(hh + 1) * 64, b * 256 + kt * 128: b * 256 + (kt + 1) * 128],
                            qs_T[hh * 64:(hh + 1) * 64, b * 256:(b + 1) * 256],
                            start=True, stop=True)
                    pTs = sb.tile([128, 2, 256], bf16, tag="pTs", name=f"pTS_{b}_{hh}")
                    nc.scalar.activation(pTs[:], pss[:], AF.Exp, scale=0.125)
                    pso = psO.tile([65, 256], f32, tag="pso", name=f"psoS_{b}_{hh}")
                    for kt in range(2):
                        nc.tensor.matmul(pso[:], vaug_s[:, b, kt, hh, 0:65], pTs[:, kt, :],
                                         start=(kt == 0), stop=(kt == 1))
                    rinv = sb.tile([1, 256], f32, tag="rinv", name=f"riS_{b}_{hh}")
                    nc.vector.reciprocal(rinv[:], pso[64:65, :])
                    rb = sb.tile([64, 256], f32, tag="rb", name=f"rbS_{b}_{hh}")
                    nc.gpsimd.partition_broadcast(rb[:], rinv[:])
                    oslice = o_self[hh * 64:(hh + 1) * 64, b * 256:(b + 1) * 256]
                    nc.vector.tensor_tensor(oslice, pso[0:64, :], rb[:], ALU.mult)
                    nc.vector.tensor_scalar_add(oslice, oslice, bv_sb[hh * 64:(hh + 1) * 64, :])

            ag1_in = dram.tile([128, T], bf16)
            ag1_out = dram.tile([NC, 128, T], bf16)
            nc.sync.dma_start(ag1_in[:], o_self[:])
            nc.gpsimd.collective_compute(
                "AllGather", ALU.bypass, replica_groups=[list(range(NC))],
                ins=[ag1_in[:].opt()], outs=[ag1_out[:].opt()])

            # ---- K/V projection for batch 0 chunks (overlaps AG1 + sa_out path) ----
            def kv_chunk(c):
                ctx_T = stream.tile([128, FO, SCH], fp8, tag="ctx_T", name=f"ctxT_{c}")
                nc.sync.dma_start(ctx_T[:], ctx_r[:, :, c * SCH:(c + 1) * SCH])
                psk = psA.tile([128, 512], f32, tag="ps512", name=f"ps_k{c}")
                for fp in range(4):
                    nc.tensor.matmul(psk[:], ck_sb[:, 2 * fp:2 * fp + 2, :],
                                     ctx_T[:, 2 * fp:2 * fp + 2, :],
                                     start=(fp == 0), stop=(fp == 3), perf_mode=DR)
                nc.scalar.activation(kfull[:, c * SCH:(c + 1) * SCH], psk[:],
                                     AF.Identity, bias=ckb_sb[:])
                psv = psA.tile([128, 512], f32, tag="ps512", name=f"ps_v{c}")
                for kt in range(4):
                    for fp in range(4):
                        nc.tensor.matmul(psv[:, kt * 128:(kt + 1) * 128],
                                         ctx_T[:, 2 * fp:2 * fp + 2, kt * 128:(kt + 1) * 128],
                                         cv_sb[:, 2 * fp:2 * fp + 2, :],
                                         start=(fp == 0), stop=(fp == 3), perf_mode=DR)
                nc.vector.tensor_copy(
                    vaug[:, :, c * 4:c * 4 + 4, 0:64],
                    psv[:].rearrange("p (kt h f) -> p h kt f", kt=4, h=2, f=64))

            for c in range(NSC):
                kv_chunk(c)

            o_full = fm.tile([128, FO, T], bf16, tag="fmT", name="o_full")
            for s in range(NC):
                nc.sync.dma_start(o_full[:, s, :], ag1_out[s])

            # ---- P3: sa_out projection (replicated) + residual -> x_sb (bf16) ----
            xn2_dram = dram.tile([T, D], bf16)
            for oc in range(2):
                saow_c = wstream.tile([128, FO, 512], bf16, tag="wc", name=f"saow_c{oc}")
                nc.sync.dma_start(saow_c[:], saow_r[:, :, oc * 512:(oc + 1) * 512])
                for tt in range(FO):
                    ps = psA.tile([128, 512], f32, tag="ps512", name=f"ps_x{oc}_{tt}")
                    for s in range(FO):
                        nc.tensor.matmul(ps[:], o_full[:, s, tt * 128:(tt + 1) * 128],
                                         saow_c[:, s, :],
                                         start=(s == 0), stop=(s == FO - 1))
                    qres = rstream.tile([128, 512], f32, tag="qres", name=f"qres{oc}_{tt}")
                    nc.sync.dma_start(qres[:], queries[tt * 128:(tt + 1) * 128, oc * 512:(oc + 1) * 512])
                    xs = x_sb[:, tt, oc * 512:(oc + 1) * 512]
                    nc.vector.tensor_tensor(xs, ps[:], qres[:], ALU.add)
                    nc.vector.tensor_tensor(xs, xs, saob_bc[:, oc * 512:(oc + 1) * 512], ALU.add)

            # ---- P4: LN2(x) -> xn2_T -> cross q (fp8) ----
            ln_tiles(lambda tt: x_sb[:, tt, :], lambda tt, xn: nc.sync.dma_start(
                xn2_dram[tt * 128:(tt + 1) * 128, :], xn[:]), FO, "ln2")
            xn2_T = fm.tile([128, FO, T], bf16, tag="fmT", name="xn2_T")
            for fo in range(FO):
                nc.sync.dma_start_transpose(xn2_T[:, fo, :], xn2_dram[:, fo * 128:(fo + 1) * 128])
                nc.vector.tensor_scalar(xn2_T[:, fo, :], xn2_T[:, fo, :],
                                        scalar1=g2[:, fo, :], scalar2=be2[:, fo, :],
                                        op0=ALU.mult, op1=ALU.add)
            for tc2 in range(2):
                ps = psA.tile([128, 512], f32, tag="ps512", name=f"ps_q2{tc2}")
                for f in range(FO):
                    nc.tensor.matmul(ps[:], cq_sb[:, f, :], xn2_T[:, f, tc2 * 512:(tc2 + 1) * 512],
                                     start=(f == 0), stop=(f == FO - 1))
                nc.scalar.activation(q2_sb[:, tc2 * 512:(tc2 + 1) * 512], ps[:],
                                     AF.Identity, bias=cqb_sb[:])

            # ---- K/V projection for batches 1-3 ----
            for c in range(NSC, NCH):
                kv_chunk(c)

            # ---- P5: cross-attn score/AV stream, per batch; tails overlap ----
            o_cross = big.tile([128, T], bf16, tag="o_cross")
            oc_full = fm.tile([128, FO, T], bf16, tag="fmT", name="oc_full")
            x3_own = big.tile([128, D], f32, tag="x3_own")

            ag2_ins, ag2_outs = [], []
            for b in range(B):
                ag2_ins.append(dram.tile([128, Q], bf16, tag=f"ag2i_{b}", name=f"ag2i_{b}"))
                ag2_outs.append(dram.tile([NC, 128, Q], bf16, tag=f"ag2o_{b}", name=f"ag2o_{b}"))

            for b in range(B):
                pso_h = [psO.tile([65, 256], f32, tag="pso", name=f"psoC_{b}_{h}")
                         for h in range(HPC)]
                for i in range(GPB):
                    g0 = b * 64 + 2 * i
                    tok0 = g0 * 128
                    for h in range(HPC):
                        pss = psX.tile([128, 2, 256], f32, tag="psx", name=f"pssC_{b}_{i}_{h}")
                        for j in range(2):
                            nc.tensor.matmul(
                                pss[:, j, :],
                                kfull[h * 64:(h + 1) * 64, tok0 + j * 128: tok0 + (j + 1) * 128],
                                q2_sb[h * 64:(h + 1) * 64, b * 256:(b + 1) * 256],
                                start=True, stop=True)
                        pT = pts.tile([128, 2, 256], fp8, tag="pT", name=f"pT_{b}_{i}_{h}")
                        nc.scalar.activation(pT[:], pss[:], AF.Exp, scale=0.125 / KSC)
                        nc.tensor.matmul(pso_h[h][:], vaug[:, h, g0:g0 + 2, 0:65],
                                         pT[:],
                                         start=(i == 0), stop=(i == GPB - 1), perf_mode=DR)
                for h in range(HPC):
                    rinv = sb.tile([1, 256], f32, tag="rinv", name=f"riC_{b}_{h}")
                    nc.vector.reciprocal(rinv[:], pso_h[h][64:65, :])
                    rb = sb.tile([64, 256], f32, tag="rb", name=f"rbC_{b}_{h}")
                    nc.gpsimd.partition_broadcast(rb[:], rinv[:])
                    oslice = o_cross[h * 64:(h + 1) * 64, b * 256:(b + 1) * 256]
                    nc.vector.tensor_tensor(oslice, pso_h[h][0:64, :], rb[:], ALU.mult)
                    nc.vector.tensor_scalar_add(oslice, oslice, cvb_sb[h * 64:(h + 1) * 64, :])

                nc.sync.dma_start(ag2_ins[b][:], o_cross[:, b * 256:(b + 1) * 256])
                nc.gpsimd.collective_compute(
                    "AllGather", ALU.bypass, replica_groups=[list(range(NC))],
                    ins=[ag2_ins[b][:].opt()], outs=[ag2_outs[b][:].opt()])
                for s in range(NC):
                    nc.sync.dma_start(oc_full[:, s, b * 256:(b + 1) * 256], ag2_outs[b][s])

                # co-projection + residual for this batch's two token tiles
                for oc in range(2):
                    cow_c = wstream.tile([128, FO, 512], bf16, tag="wc", name=f"cow_c{b}_{oc}")
                    nc.sync.dma_start(cow_c[:], cow_r[:, :, oc * 512:(oc + 1) * 512])
                    for tt in (2 * b, 2 * b + 1):
                        ps = psS.tile([128, 512], f32, tag="pss", name=f"ps_co{tt}_{oc}")
                        for s in range(FO):
                            nc.tensor.matmul(ps[:], oc_full[:, s, tt * 128:(tt + 1) * 128],
                                             cow_c[:, s, :],
                                             start=(s == 0), stop=(s == FO - 1))
                        xs = x_sb[:, tt, oc * 512:(oc + 1) * 512]
                        nc.vector.tensor_tensor(xs, ps[:], xs, ALU.add)
                        nc.vector.tensor_tensor(xs, xs, cob_bc[:, oc * 512:(oc + 1) * 512], ALU.add)

                # accumulate own-token selection (one-hot) for this batch
                for oc in range(2):
                    ps = psS.tile([128, 512], f32, tag="pss", name=f"ps_sel{b}_{oc}")
                    for k, tt in enumerate((2 * b, 2 * b + 1)):
                        nc.tensor.matmul(ps[:], psel_sb[:, tt, :],
                                         x_sb[:, tt, oc * 512:(oc + 1) * 512],
                                         start=(k == 0), stop=(k == 1))
                    dst = x3_own[:, oc * 512:(oc + 1) * 512]
                    if b == 0:
                        nc.vector.tensor_copy(dst, ps[:])
                    else:
                        nc.vector.tensor_tensor(dst, dst, ps[:], ALU.add)

            # ---- P6: LN3 on own tokens -> xn3_T (PE transpose) ----
            xn3_keep = [None]
            ln_tiles(lambda tt: x3_own[:],
                     lambda tt, xn: xn3_keep.__setitem__(0, xn), 1, "ln3")
            xn3 = xn3_keep[0]
            xn3_T = big.tile([128, FO, 128], bf16, tag="xn3_T")
            for fo in range(FO):
                ptr = psA.tile([128, 128], bf16, tag="ps512", name=f"ptr3_{fo}")
                nc.tensor.transpose(ptr[:], xn3[:, fo * 128:(fo + 1) * 128], ident[:])
                nc.vector.tensor_scalar(xn3_T[:, fo, :], ptr[:],
                                        scalar1=g3[:, fo, :], scalar2=be3[:, fo, :],
                                        op0=ALU.mult, op1=ALU.add)

            # ---- P7: FFN on own tokens (dff-major h, no transposes) ----
            h_T = big.tile([128, 32, 128], bf16, tag="h_T")
            for wg in range(8):
                w1c = wstream.tile([128, FO, 512], bf16, tag="wc", name=f"w1c{wg}")
                nc.sync.dma_start(w1c[:], w1_r[:, :, wg * 512:(wg + 1) * 512])
                for dd in range(4):
                    dc = wg * 4 + dd
                    ph = psA.tile([128, 128], f32, tag="ps512", name=f"ps_h{dc}")
                    for f in range(FO):
                        nc.tensor.matmul(ph[:], w1c[:, f, dd * 128:(dd + 1) * 128],
                                         xn3_T[:, f, :],
                                         start=(f == 0), stop=(f == FO - 1))
                    nc.scalar.activation(h_T[:, dc, :], ph[:], AF.Gelu,
                                         bias=b1_sb[:, dc:dc + 1])
            out_sb = big.tile([128, D], f32, tag="out_sb")
            for oc in range(2):
                psy = psS.tile([128, 512], f32, tag="pss", name=f"ps_y{oc}")
                for dg in range(4):
                    w2c = wstream.tile([128, FO, 512], bf16, tag="wc", name=f"w2c{oc}_{dg}")
                    nc.sync.dma_start(w2c[:], w2_r[:, dg * FO:(dg + 1) * FO, oc * 512:(oc + 1) * 512])
                    for j in range(FO):
                        dt_ = dg * FO + j
                        nc.tensor.matmul(psy[:], h_T[:, dt_, :], w2c[:, j, :],
                                         start=(dt_ == 0), stop=(dt_ == 31))
                ys = out_sb[:, oc * 512:(oc + 1) * 512]
                nc.vector.tensor_tensor(ys, psy[:], x3_own[:, oc * 512:(oc + 1) * 512], ALU.add)
                nc.vector.tensor_tensor(ys, ys, b2_bc[:, oc * 512:(oc + 1) * 512], ALU.add)
            nc.sync.dma_start(out[:], out_sb[:])

    nc.compile()
    return nc


def _pack_ln(v):
    return np.ascontiguousarray(np.asarray(v, dtype=np.float32).reshape(FO, 128).T)


def _get_nc():
    if "nc" not in _CACHE:
        _CACHE["nc"] = _build()
    return _CACHE["nc"]


def _bf(a):
    return np.ascontiguousarray(a).astype(BF16NP)


def _f8(a):
    return np.clip(np.ascontiguousarray(a), -240.0, 240.0).astype(FP8NP)


def kernel(**inputs):
    nc = _get_nc()
    inp = {k: np.asarray(v) for k, v in inputs.items()}

    queries = np.ascontiguousarray(inp["queries"].reshape(T, D).astype(np.float32))
    ctx8 = _f8(inp["context"].reshape(B * S, D).T)
    sa_in_w = inp["sa_in_w"]; sa_in_b = inp["sa_in_b"]
    b1v = np.asarray(inp["b1"], np.float32).reshape(32, 128).T
    shared = {
        "queries": queries, "ctx": ctx8,
        "ln1g": _pack_ln(inp["ln1_g"]), "ln1b": _pack_ln(inp["ln1_b"]),
        "ln2g": _pack_ln(inp["ln2_g"]), "ln2b": _pack_ln(inp["ln2_b"]),
        "ln3g": _pack_ln(inp["ln3_g"]), "ln3b": _pack_ln(inp["ln3_b"]),
        "saow": _bf(inp["sa_out_w"].T), "saob": _bf(inp["sa_out_b"]),
        "cow": _bf(inp["co_w"].T / KSC), "cob": _bf(inp["co_b"]),
        "w1": _bf(inp["w1"].T), "b1": np.ascontiguousarray(b1v),
        "w2": _bf(inp["w2"].T), "b2": np.asarray(inp["b2"], np.float32),
        "ident": np.eye(128, dtype=BF16NP),
    }
    in_maps = []
    eye = np.eye(128, dtype=np.float32)
    for c in range(NC):
        r = slice(c * 128, (c + 1) * 128)
        psel = np.zeros((128, FO, 128), np.float32)
        psel[:, c, :] = eye
        m = dict(shared)
        m.update({
            "wq": _bf(sa_in_w[0 * D:1 * D][r].T), "bq": np.asarray(sa_in_b[0 * D:1 * D][r], np.float32),
            "wk": _bf(sa_in_w[1 * D:2 * D][r].T), "bk": np.asarray(sa_in_b[1 * D:2 * D][r], np.float32),
            "wv": _bf(sa_in_w[2 * D:3 * D][r].T), "bv": np.asarray(sa_in_b[2 * D:3 * D][r], np.float32),
            "cqw": _bf(inp["cq_w"][r].T), "cqb": np.asarray(inp["cq_b"][r], np.float32),
            "ckw": _f8(inp["ck_w"][r].T * KSC), "ckb": np.asarray(inp["ck_b"][r], np.float32) * KSC,
            "cvw": _f8(inp["cv_w"][r].T * KSC), "cvb": np.asarray(inp["cv_b"][r], np.float32) * KSC,
            "psel": _bf(psel),
        })
        in_maps.append(m)

    res = run_bass_kernel_spmd(nc, in_maps, core_ids=list(range(NC)),
                               **_CACHE.get("run_kwargs", {}))
    _CACHE["last_result"] = res
    out = np.concatenate([np.asarray(res.results[c]["out"]) for c in range(NC)], axis=0)
    return out.reshape(B, Q, D).astype(np.float32)
